# revision 1
# baseline (speedup 1.0000x reference)
"""GATv2 (3-layer, 4-head) on 8 Trainium2 NeuronCores — Bass/Tile SPMD kernel.

Sharding: destination-node partition (graph parallel). Core c owns dst nodes
[c*NPC, (c+1)*NPC) in NBLK blocks of BLK. Edges (incl. mean-filled
self-loops) are bucketed by dst block; all cores run one shared SPMD
program over padded, per-core index data.

Per layer:
  1. sharded node matmuls xl = h@Wl, xr = h@Wr (own 2500 nodes)
  2. AllGather of the xl table (only collective; xr stays local)
  3. per dst-block: dma_gather xl[src] and xr[dst] rows,
     z = ee + xl_g + xr_g assembled in PSUM via matmuls,
     leaky_relu on DVE, att-dot + per-head reduce, exp on ScalarE
     (softmax max-shift dropped: alpha is shift-invariant, logits are O(1)),
     unnormalized scatter out += Ind^T @ (w * xl_g) and denom += Ind^T @ w
     via one-hot indicator matmuls (indicators built on-device by is_equal),
     then normalize by 1/denom, head-mean, bias, outer leaky_relu.
"""
import sys

sys.path.insert(0, "/opt/trn_rl_repo")
from contextlib import ExitStack

import numpy as np
import concourse.bacc as bacc
import concourse.mybir as mybir
import concourse.tile as tile
from concourse.bass_utils import run_bass_kernel_spmd
from concourse.library_config import mlp

f32 = mybir.dt.float32
i16 = mybir.dt.int16
ALU = mybir.AluOpType
AF = mybir.ActivationFunctionType

H = 4
D = 128
HD = H * D
F_IN = 128
NEG = 0.2
N_LAYERS = 3
C = 8
TILE = 128

# full-problem dims (overridable for small-scale sim tests)
DIMS = dict(N=20000, NPC=2500, BLK=125, NBLK=20)

_BUILD_CACHE = {}


# ----------------------------------------------------------------- host prep
def _pack_idxs(il):
    n = len(il)
    a = np.zeros((128, n // 16), np.int16)
    base = il.reshape(n // 16, 16).T
    for g in range(8):
        a[g * 16:(g + 1) * 16] = base
    return a


def _build_shards(edge_index, edge_attr, dims=DIMS):
    N, NPC, BLK, NBLK = dims["N"], dims["NPC"], dims["BLK"], dims["NBLK"]
    src = np.asarray(edge_index[0], np.int64)
    dst = np.asarray(edge_index[1], np.int64)
    ea = np.asarray(edge_attr, np.float32)

    ea_sum = np.zeros((N, 2), np.float32)
    np.add.at(ea_sum, dst, ea)
    cnt = np.bincount(dst, minlength=N).astype(np.float32)
    loop_attr = ea_sum / np.maximum(cnt, 1.0)[:, None]

    fsrc = np.concatenate([src, np.arange(N, dtype=np.int64)])
    fdst = np.concatenate([dst, np.arange(N, dtype=np.int64)])
    ffea = np.concatenate([ea, loop_attr], axis=0)

    key = fdst // NPC * NBLK + (fdst % NPC) // BLK
    order = np.argsort(key, kind="stable")
    kb = key[order]
    bounds = np.searchsorted(kb, np.arange(C * NBLK + 1))
    max_edges = int(np.max(np.diff(bounds)))
    tpb = (max_edges + TILE - 1) // TILE
    epb = tpb * TILE
    ec = NBLK * epb

    shards = []
    for c in range(C):
        s_src = np.zeros(ec, np.int16)
        s_dstloc = np.zeros(ec, np.int16)
        s_fea = np.zeros((ec, 2), np.float32)
        s_valid = np.zeros(ec, bool)
        for b in range(NBLK):
            k = c * NBLK + b
            el = order[bounds[k]:bounds[k + 1]]
            o = b * epb
            n = len(el)
            s_src[o:o + n] = fsrc[el].astype(np.int16)
            s_dstloc[o:o + n] = (fdst[el] - c * NPC).astype(np.int16)
            s_fea[o:o + n] = ffea[el]
            s_valid[o:o + n] = True
        t_ids = np.arange(ec) // TILE
        rel = s_dstloc.astype(np.float32) - (t_ids // tpb) * BLK
        rel[~s_valid] = -1.0  # padding matches no indicator column
        dstrel_f = np.ascontiguousarray(rel.reshape(ec // TILE, TILE).T)
        shards.append(dict(
            src_pk=_pack_idxs(s_src),
            dst_pk=_pack_idxs(np.maximum(s_dstloc, 0)),
            feaT=np.ascontiguousarray(s_fea.T),
            dstrel=dstrel_f.astype(np.float32),
        ))
    return shards, tpb


# --------------------------------------------------------------- device build
def _build(tpb, nzb, dims=DIMS, compile=True):
    key = (tpb, nzb, tuple(sorted(dims.items())))
    if key in _BUILD_CACHE:
        return _BUILD_CACHE[key]
    N, NPC, BLK, NBLK = dims["N"], dims["NPC"], dims["BLK"], dims["NBLK"]
    nz_bf, nz_bl, nz_br, nz_bo = nzb
    epb = tpb * TILE
    ec = NBLK * epb

    nc = bacc.Bacc("TRN2", target_bir_lowering=False, debug=False, num_devices=C)
    d_xT = nc.dram_tensor("xT", [F_IN, NPC], f32, kind="ExternalInput")
    d_feaT = nc.dram_tensor("feaT", [2, ec], f32, kind="ExternalInput")
    d_srcpk = nc.dram_tensor("src_pk", [128, ec // 16], i16, kind="ExternalInput")
    d_dstpk = nc.dram_tensor("dst_pk", [128, ec // 16], i16, kind="ExternalInput")
    d_dstrel = nc.dram_tensor("dstrel", [128, ec // TILE], f32, kind="ExternalInput")
    d_eye = nc.dram_tensor("eye", [128, 128], f32, kind="ExternalInput")
    d_iorow = nc.dram_tensor("iorow", [128, BLK], f32, kind="ExternalInput")
    d_Wf = nc.dram_tensor("Wf", [F_IN, D], f32, kind="ExternalInput")
    d_Wl = nc.dram_tensor("Wl", [D, HD], f32, kind="ExternalInput")
    d_Wr = nc.dram_tensor("Wr", [D, HD], f32, kind="ExternalInput")
    d_We = nc.dram_tensor("We", [2, HD], f32, kind="ExternalInput")
    d_attb = nc.dram_tensor("att_b", [128, HD], f32, kind="ExternalInput")
    d_bf = nc.dram_tensor("bf_col", [128, 1], f32, kind="ExternalInput")
    d_blb = nc.dram_tensor("bl_b", [128, HD], f32, kind="ExternalInput")
    d_brb = nc.dram_tensor("br_b", [128, HD], f32, kind="ExternalInput")
    d_bob = nc.dram_tensor("bo_b", [128, D], f32, kind="ExternalInput")
    d_out = nc.dram_tensor("hout", [NPC, D], f32, kind="ExternalOutput")

    with tile.TileContext(nc) as tc, ExitStack() as ex:
        cst = ex.enter_context(tc.tile_pool(name="cst", bufs=1))
        dram = ex.enter_context(tc.tile_pool(name="dram", bufs=1, space="DRAM"))
        ps512 = ex.enter_context(tc.tile_pool(name="ps512", bufs=4, space="PSUM"))
        psO = ex.enter_context(tc.tile_pool(name="psO", bufs=2, space="PSUM"))
        psD = ex.enter_context(tc.tile_pool(name="psD", bufs=2, space="PSUM"))
        gb1 = ex.enter_context(tc.tile_pool(name="gb1", bufs=2))
        gb2 = ex.enter_context(tc.tile_pool(name="gb2", bufs=2))
        scr = ex.enter_context(tc.tile_pool(name="scr", bufs=3))
        blkp = ex.enter_context(tc.tile_pool(name="blkp", bufs=2))
        evp = ex.enter_context(tc.tile_pool(name="evp", bufs=3))
        feap = ex.enter_context(tc.tile_pool(name="feap", bufs=2))

        nc.gpsimd.load_library(mlp)

        def ld(dt, shape, dtype=f32):
            t = cst.tile(shape, dtype, name=f"sb_{dt.name}")
            nc.sync.dma_start(t[:], dt[:])
            return t

        eye = ld(d_eye, [128, 128])
        iorow = ld(d_iorow, [128, BLK])
        Wf = ld(d_Wf, [F_IN, D])
        Wl = ld(d_Wl, [D, HD])
        Wr = ld(d_Wr, [D, HD])
        We = ld(d_We, [2, HD])
        attb = ld(d_attb, [128, HD])
        xT = ld(d_xT, [F_IN, NPC])
        srcpk = ld(d_srcpk, [128, ec // 16], i16)
        dstpk = ld(d_dstpk, [128, ec // 16], i16)
        dstrel = ld(d_dstrel, [128, ec // TILE])
        bf = ld(d_bf, [128, 1]) if nz_bf else None
        blb = ld(d_blb, [128, HD]) if nz_bl else None
        brb = ld(d_brb, [128, HD]) if nz_br else None
        bob = ld(d_bob, [128, D]) if nz_bo else None

        hT = cst.tile([128, NPC], f32, name="hT")
        agins = [dram.tile([NPC, HD], f32, name=f"agin{i}")
                 for i in range(N_LAYERS)]
        agouts = [dram.tile([N, HD], f32, addr_space="Shared", name=f"agout{i}")
                  for i in range(N_LAYERS)]
        xr_d = dram.tile([NPC, HD], f32)

        # ---- layer-0 features, feature-major: h0T = Wf.T @ xT (+ bf)
        CH = min(NPC, 500)
        assert NPC % CH == 0
        for j in range(NPC // CH):
            ps = ps512.tile([128, CH], f32, tag="ps512")
            nc.tensor.matmul(ps[:], Wf[:], xT[:, j * CH:(j + 1) * CH],
                             start=True, stop=True)
            dst = hT[:, j * CH:(j + 1) * CH]
            if nz_bf:
                nc.vector.tensor_scalar_add(dst, ps[:], bf[:])
            else:
                nc.vector.tensor_copy(dst, ps[:])

        for L in range(N_LAYERS):
            agin, agout = agins[L], agouts[L]
            # ---- node matmuls (own shard) -> xl to agin, xr to xr_d
            for m in range(NBLK):
                lh = hT[:, m * BLK:(m + 1) * BLK]
                psl = ps512.tile([BLK, HD], f32, tag="ps512")
                nc.tensor.matmul(psl[:], lh, Wl[:], start=True, stop=True)
                xle = evp.tile([BLK, HD], f32, tag="ev")
                if nz_bl:
                    nc.vector.tensor_add(xle[:], psl[:], blb[:BLK, :])
                else:
                    nc.vector.tensor_copy(xle[:], psl[:])
                nc.sync.dma_start(agin[m * BLK:(m + 1) * BLK, :], xle[:])
                psr = ps512.tile([BLK, HD], f32, tag="ps512")
                nc.tensor.matmul(psr[:], lh, Wr[:], start=True, stop=True)
                xre = evp.tile([BLK, HD], f32, tag="ev")
                if nz_br:
                    nc.vector.tensor_add(xre[:], psr[:], brb[:BLK, :])
                else:
                    nc.vector.tensor_copy(xre[:], psr[:])
                nc.sync.dma_start(xr_d[m * BLK:(m + 1) * BLK, :], xre[:])

            nc.gpsimd.collective_compute(
                "AllGather", ALU.bypass,
                replica_groups=[list(range(C))],
                ins=[agin.opt()], outs=[agout.opt()],
            )

            # ---- edge phase, per dst block
            for b in range(NBLK):
                e0 = b * epb
                GC = 4  # tiles per gather call (512 idxs: SWDGE ring limit)
                xlg = gb1.tile([128, tpb, HD], f32, tag="xlg")
                xrg = gb2.tile([128, tpb, HD], f32, tag="xrg")
                for g0 in range(0, tpb, GC):
                    g1 = min(g0 + GC, tpb)
                    ne = (g1 - g0) * TILE
                    c0 = (e0 + g0 * TILE) // 16
                    nc.gpsimd.dma_gather(xlg[:, g0:g1, :], agout[:],
                                         srcpk[:, c0:c0 + ne // 16],
                                         ne, ne, HD)
                    nc.gpsimd.dma_gather(xrg[:, g0:g1, :], xr_d[:],
                                         dstpk[:, c0:c0 + ne // 16],
                                         ne, ne, HD)
                feaT = feap.tile([2, epb], f32, tag="feaT")
                nc.sync.dma_start(feaT[:], d_feaT[:, e0:e0 + epb])
                lgb = blkp.tile([128, tpb, H], f32, tag="lgb")
                indb = blkp.tile([128, tpb, BLK], f32, tag="indb")
                for t in range(tpb):
                    nc.vector.tensor_scalar(
                        indb[:, t, :], iorow[:],
                        dstrel[:, b * tpb + t:b * tpb + t + 1], None,
                        ALU.is_equal)
                    zp = ps512.tile([128, HD], f32, tag="ps512")
                    et = e0 + t * TILE
                    nc.tensor.matmul(zp[:], feaT[:, t * TILE:(t + 1) * TILE],
                                     We[:], start=True, stop=False)
                    nc.tensor.matmul(zp[:], eye[:], xlg[:, t, :],
                                     start=False, stop=False)
                    nc.tensor.matmul(zp[:], eye[:], xrg[:, t, :],
                                     start=False, stop=True)
                    lz0 = scr.tile([128, HD], f32, tag="lz0")
                    nc.scalar.activation(lz0[:], zp[:], AF.Copy, scale=NEG)
                    lz = scr.tile([128, HD], f32, tag="lz")
                    nc.vector.tensor_max(lz[:], lz0[:], zp[:])
                    y = scr.tile([128, HD], f32, tag="y")
                    nc.vector.tensor_mul(y[:], lz[:], attb[:])
                    nc.vector.tensor_reduce(
                        lgb[:, t, :], y[:].rearrange("p (h d) -> p h d", h=H),
                        axis=mybir.AxisListType.X, op=ALU.add)
                web = blkp.tile([128, tpb, H], f32, tag="web")
                nc.scalar.activation(web[:], lgb[:], AF.Exp)
                outp = psO.tile([BLK, HD], f32, tag="psO")
                denp = psD.tile([BLK, H], f32, tag="psD")
                for t in range(tpb):
                    msg = scr.tile([128, HD], f32, tag="msg")
                    for hh in range(H):
                        nc.vector.tensor_scalar_mul(
                            msg[:, hh * D:(hh + 1) * D],
                            xlg[:, t, hh * D:(hh + 1) * D],
                            web[:, t, hh:hh + 1])
                    nc.tensor.matmul(outp[:], indb[:, t, :], msg[:],
                                     start=(t == 0), stop=(t == tpb - 1))
                    nc.tensor.matmul(denp[:], indb[:, t, :], web[:, t, :],
                                     start=(t == 0), stop=(t == tpb - 1))
                invd = blkp.tile([BLK, H], f32, tag="invd")
                nc.vector.reciprocal(invd[:], denp[:])
                # fold the head-mean 1/H into the normalizer
                nc.vector.tensor_scalar_mul(invd[:], invd[:], 1.0 / H)
                o = blkp.tile([BLK, HD], f32, tag="o")
                for hh in range(H):
                    nc.vector.tensor_scalar_mul(
                        o[:, hh * D:(hh + 1) * D],
                        outp[:, hh * D:(hh + 1) * D],
                        invd[:, hh:hh + 1])
                s01 = blkp.tile([BLK, D], f32, tag="s01")
                nc.vector.tensor_add(s01[:], o[:, 0:D], o[:, D:2 * D])
                s23 = blkp.tile([BLK, D], f32, tag="s23")
                nc.vector.tensor_add(s23[:], o[:, 2 * D:3 * D], o[:, 3 * D:4 * D])
                sm = blkp.tile([BLK, D], f32, tag="sm")
                nc.vector.tensor_add(sm[:], s01[:], s23[:])
                if nz_bo:
                    nc.vector.tensor_add(sm[:], sm[:], bob[:BLK, :])
                hb = blkp.tile([BLK, D], f32, tag="hb")
                nc.vector.scalar_tensor_tensor(
                    hb[:], sm[:], 0.01, sm[:], ALU.mult, ALU.max)
                if L == N_LAYERS - 1:
                    nc.sync.dma_start(d_out[b * BLK:(b + 1) * BLK, :], hb[:])
                else:
                    tp = ps512.tile([128, BLK], f32, tag="ps512")
                    nc.tensor.transpose(tp[:], hb[:], eye[:BLK, :BLK])
                    nc.vector.tensor_copy(hT[:, b * BLK:(b + 1) * BLK], tp[:])

    if compile:
        nc.compile()
    _BUILD_CACHE[key] = nc
    return nc


# ------------------------------------------------------------------ in_maps
def make_in_maps(inputs, dims=DIMS):
    N, NPC, BLK = dims["N"], dims["NPC"], dims["BLK"]
    x = np.asarray(inputs["x"], np.float32)
    Wf = np.ascontiguousarray(np.asarray(inputs["Wf"], np.float32))
    bf = np.asarray(inputs["bf"], np.float32)
    Wl = np.ascontiguousarray(np.asarray(inputs["Wl"], np.float32))
    bl = np.asarray(inputs["bl"], np.float32)
    Wr = np.ascontiguousarray(np.asarray(inputs["Wr"], np.float32))
    br = np.asarray(inputs["br"], np.float32)
    We = np.ascontiguousarray(np.asarray(inputs["We"], np.float32))
    att = np.asarray(inputs["att"], np.float32)
    bias_out = np.asarray(inputs["bias_out"], np.float32)

    shards, tpb = _build_shards(inputs["edge_index"], inputs["edge_attr"], dims)
    nzb = (bool(bf.any()), bool(bl.any()), bool(br.any()), bool(bias_out.any()))

    common = dict(
        eye=np.eye(128, dtype=np.float32),
        iorow=np.tile(np.arange(BLK, dtype=np.float32), (128, 1)),
        Wf=Wf, Wl=Wl, Wr=Wr, We=We,
        att_b=np.tile(att.reshape(1, HD), (128, 1)).astype(np.float32),
        bf_col=np.ascontiguousarray(bf.reshape(D, 1)),
        bl_b=np.tile(bl.reshape(1, HD), (128, 1)).astype(np.float32),
        br_b=np.tile(br.reshape(1, HD), (128, 1)).astype(np.float32),
        bo_b=np.tile(bias_out.reshape(1, D), (128, 1)).astype(np.float32),
    )
    in_maps = []
    for c in range(C):
        sh = shards[c]
        m = dict(common)
        m["xT"] = np.ascontiguousarray(x[c * NPC:(c + 1) * NPC].T)
        m["feaT"] = sh["feaT"]
        m["src_pk"] = sh["src_pk"]
        m["dst_pk"] = sh["dst_pk"]
        m["dstrel"] = sh["dstrel"]
        in_maps.append(m)
    return in_maps, tpb, nzb


# -------------------------------------------------------------- bench hooks
def build_for_inputs(inputs):
    in_maps, tpb, nzb = make_in_maps(inputs, DIMS)
    nc = _build(tpb, nzb, DIMS)
    return nc, in_maps


def assemble_output(outs, out_names):
    NPC = DIMS["NPC"]
    got = np.asarray(outs[out_names.index("hout")]).reshape(C, NPC, -1)
    return got.reshape(C * NPC, -1).astype(np.float32)


# -------------------------------------------------------------------- kernel
def kernel(**inputs):
    in_maps, tpb, nzb = make_in_maps(inputs, DIMS)
    nc = _build(tpb, nzb, DIMS)
    res = run_bass_kernel_spmd(nc, in_maps, list(range(C)))
    NPC = DIMS["NPC"]
    return np.concatenate([res.results[c]["hout"] for c in range(C)], axis=0)


if __name__ == "__main__":
    nc = _build(10, (False, False, False, False), DIMS, compile=False)
    n_inst = sum(len(f.blocks[0].instructions) for f in nc.m.functions)
    print("trace-only build OK")

